# revision 50
# baseline (speedup 1.0000x reference)
"""Trainium2 Bass kernel for GAT-style attention score computation.

Math (see reference):
    s_src = X @ a[:F];  s_dst = X @ a[F:]
    e[i, j] = leaky_relu(s_src[i] + s_dst[j], alpha=0.2)

Sharding over 8 NeuronCores: row-shard the N x N output (1024 rows/core).
No collective (a 4 KB AllGather measured ~75 us of latency/skew): each
core receives a replicated bf16 copy of X^T and computes the full s_dst
row vector itself with native-rate bf16 matmuls whose stationary operand
is a_dst replicated along the free dim, so the PSUM result is s_dst
broadcast across all 128 partitions (d_bcast, cast to bf16 in SBUF).

s_src also runs on TensorE: the transposed local shard block
xlocT[:, t*128:(t+1)*128] as stationary against an a_src-replicated
moving tile gives out[p, j] = s_src[t*128+p] for every j -- the
activation bias column for sub-row block t, with no DVE matvec chain.

Main loop (half = 4096 output columns; d_bcast h0 first, h1 built after
the first three h0 pairs): 8 sub-rows t per half, local row r = t*128+p,
paired into 2 MB output DMAs.  Within a pair one row runs on the scalar
engine (Prelu activation with per-partition bias) and one on the vector
engine (u = d+s; v = 0.2*d+0.2*s; max(u,v)), so a pair completes at
max(ACT, DVE).  DVE rows sit only in sync-ring pairs: a scalar-ring
issue would embed a DVE-semaphore wait into the ACT FIFO and stall
activations.  The first pair is split into two 1 MB writes to start the
output stream early.

Output is written in bf16 (harness gate is 2e-2 rel err; bf16 keeps it
~3e-3) and upcast to f32 on the host, halving the dominant HBM write
traffic.  Measured ~70.6 us/core (baseline f32+collective: 238.8 us).
Floor: ~21.3 MB DMA/core at ~400 GB/s + ~7 us boot + ~13 us ramp.
"""

import numpy as np

N = 8192
F = 256
NCORES = 8
ROWS = N // NCORES          # 1024 rows per core
P = 128                     # partitions
C = ROWS // P               # 8 sub-rows per partition
ALPHA = 0.2
HALF = N // 2               # 4096 columns per half
QCH = 512                   # matmul free-dim chunk (one PSUM bank)
NACT = 5                    # sub-rows on the scalar engine (rest on DVE)

_CACHE = {}


def _build():
    import concourse.bacc as bacc
    import concourse.bass as bass
    import concourse.tile as tile
    from concourse import mybir

    fp32 = mybir.dt.float32
    fp16 = mybir.dt.bfloat16  # bf16: TensorE native rate (fp16 matmuls run ~2x slower)

    nc = bacc.Bacc(
        "TRN2",
        target_bir_lowering=False,
        debug=False,
        num_devices=NCORES,
    )

    xt_dram = nc.dram_tensor("xt", [F, N], fp16, kind="ExternalInput")
    # transposed local shard, [F, 1024] fp16 (host-prepared)
    xlt_dram = nc.dram_tensor("xlocT", [F, ROWS], fp16, kind="ExternalInput")
    # av_cols[f, a] = av[a*128 + f] (host-rearranged): cols 0,1 = a_src,
    # cols 2,3 = a_dst
    avc_dram = nc.dram_tensor("av_cols", [P, 4], fp32, kind="ExternalInput")
    out_dram = nc.dram_tensor("out", [ROWS, N], fp16, kind="ExternalOutput")

    with tile.TileContext(nc) as tc:
        with (
            tc.tile_pool(name="const", bufs=1) as const_pool,
            tc.tile_pool(name="xt", bufs=2) as xt_pool,
            tc.tile_pool(name="dbc", bufs=2) as dbc_pool,
            tc.tile_pool(name="uv", bufs=4) as uv_pool,
            tc.tile_pool(name="outp", bufs=5) as out_pool,
            tc.tile_pool(name="psS", bufs=1, space=bass.MemorySpace.PSUM) as psS_pool,
            tc.tile_pool(name="psA", bufs=3, space=bass.MemorySpace.PSUM) as psA_pool,
        ):
            # ---- input loads: one 2 MB DMA per X^T half ([128, 2, 4096]:
            # kb-chunk kb of K in cols [kb*HALF:(kb+1)*HALF]); first half
            # leads on the sync ring, the rest go via the scalar ring ----
            xtAB = [
                xt_pool.tile([P, 2 * HALF], fp16, tag="xtAB", name=f"xtAB{h}")
                for h in range(2)
            ]
            xt_kb = xt_dram.ap().rearrange("(kb p) n -> p kb n", kb=2)
            a_cols = const_pool.tile([P, 4], fp32)
            nc.sync.dma_start(a_cols[:], avc_dram.ap())
            nc.sync.dma_start(
                xtAB[0][:].rearrange("p (kb n) -> p kb n", kb=2),
                xt_kb[:, :, 0:HALF],
            )
            xlt = [const_pool.tile([P, ROWS], fp16, name=f"xlt{kb}") for kb in range(2)]
            xlt_ap = xlt_dram.ap()
            for kb in range(2):
                nc.sync.dma_start(xlt[kb][:], xlt_ap[kb * P:(kb + 1) * P, :])
            nc.sync.dma_start(
                xtAB[1][:].rearrange("p (kb n) -> p kb n", kb=2),
                xt_kb[:, :, HALF:N],
            )

            # ---- stationary tiles: a_dst (lhsT) and a_src (rhs) chunks ----
            ones16 = const_pool.tile([P, P], fp16)
            nc.vector.memset(ones16[:], 1.0)
            lhsT = []
            rhs_s = []
            for kb in range(2):
                t_ = const_pool.tile([P, P], fp16, name=f"lhsT{kb}")
                nc.vector.tensor_scalar(
                    t_[:], ones16[:], a_cols[:, 2 + kb:3 + kb], None,
                    op0=mybir.AluOpType.mult,
                )
                lhsT.append(t_)
                r_ = const_pool.tile([P, C], fp16, name=f"rhs_s{kb}")
                nc.vector.tensor_scalar(
                    r_[:], ones16[:, 0:C], a_cols[:, kb:kb + 1], None,
                    op0=mybir.AluOpType.mult,
                )
                rhs_s.append(r_)

            # ---- d_bcast halves (h0 before the s_src matmuls: its input
            # lands first and it gates the main loop), s_dst replicated
            # across partitions ----
            def emit_dbc(h):
                d_ = dbc_pool.tile([P, HALF], fp16, tag="dbc", name=f"dbc{h}")
                dbc.append(d_)
                for q in range(HALF // (2 * QCH)):
                    ps = psA_pool.tile([P, 2 * QCH], fp32, tag="dps")
                    for half_q in range(2):
                        sl_p = slice(half_q * QCH, (half_q + 1) * QCH)
                        sl = slice((2 * q + half_q) * QCH, (2 * q + half_q + 1) * QCH)
                        nc.tensor.matmul(
                            ps[:, sl_p], lhsT[0][:], xtAB[h][:, sl],
                            start=True, stop=False,
                        )
                        nc.tensor.matmul(
                            ps[:, sl_p], lhsT[1][:],
                            xtAB[h][:, HALF + sl.start:HALF + sl.stop],
                            start=False, stop=True,
                        )
                    sl2 = slice(2 * q * QCH, 2 * (q + 1) * QCH)
                    nc.vector.tensor_copy(d_[:, sl2], ps[:])

            dbc = []
            emit_dbc(0)

            # ---- s_src on TensorE: block t of xlocT (stationary) against
            # the a_src-replicated tile -> out[p, j] = s_src[t*128+p] ----
            ps_s = psS_pool.tile([P, C * C], fp32)
            for t in range(C):
                nc.tensor.matmul(
                    ps_s[:, t * C:(t + 1) * C],
                    xlt[0][:, t * P:(t + 1) * P], rhs_s[0][:],
                    start=True, stop=False,
                )
                nc.tensor.matmul(
                    ps_s[:, t * C:(t + 1) * C],
                    xlt[1][:, t * P:(t + 1) * P], rhs_s[1][:],
                    start=False, stop=True,
                )
            s_src = const_pool.tile([P, C * C], fp32)
            nc.vector.tensor_copy(s_src[:], ps_s[:])
            s_srcA = const_pool.tile([P, C * C], fp32)
            nc.gpsimd.tensor_scalar(
                s_srcA[:], s_src[:], ALPHA, None, op0=mybir.AluOpType.mult
            )

            # ---- main loops: sub-rows paired into one 2 MB output DMA;
            # within a pair one row runs on ACT, the other on DVE so the
            # pair completes at max(ACT, DVE), not 2x ACT.  dbc[1] is built
            # after the first two h0 pairs so its PSUM casts don't sit in
            # the DVE FIFO ahead of the first output rows. ----
            # Per-half engine assignment: DVE rows sit only in sync-ring
            # pairs (pairs 0,2 of h0 / 1,3 of h1); scalar-ring pairs are
            # pure-ACT so their DMA issues never stall the ACT FIFO.
            ACT_ROWS_H = [{0, 2, 3, 6, 7}, {0, 2, 3, 6, 7}]
            out_view = out_dram.ap().rearrange("(c p) n -> p c n", c=C)

            def emit_pair(h, tp, split=False):
                ACT_ROWS = ACT_ROWS_H[h]
                csl = slice(h * HALF, (h + 1) * HALF)
                o = out_pool.tile([P, 2 * HALF], fp16)
                for ti in range(2):
                    t = 2 * tp + ti
                    osl = slice(ti * HALF, (ti + 1) * HALF)
                    if t in ACT_ROWS:
                        nc.scalar.activation(
                            o[:, osl],
                            dbc[h][:],
                            mybir.ActivationFunctionType.Prelu,
                            bias=s_src[:, t * C:t * C + 1],
                            scale=1.0,
                            alpha=ALPHA,
                        )
                    else:
                        u = uv_pool.tile([P, HALF], fp16, tag="u")
                        v = uv_pool.tile([P, HALF], fp16, tag="v")
                        nc.vector.tensor_scalar(
                            u[:], dbc[h][:], s_src[:, t * C:t * C + 1], None,
                            op0=mybir.AluOpType.add,
                        )
                        nc.vector.tensor_scalar(
                            v[:], dbc[h][:], ALPHA, s_srcA[:, t * C:t * C + 1],
                            op0=mybir.AluOpType.mult, op1=mybir.AluOpType.add,
                        )
                        nc.vector.tensor_tensor(
                            o[:, osl], u[:], v[:], op=mybir.AluOpType.max
                        )
                eng = nc.sync if (h * 4 + tp) % 2 == 0 else nc.scalar
                if split:
                    for ti in range(2):
                        t = 2 * tp + ti
                        osl = slice(ti * HALF, (ti + 1) * HALF)
                        e2 = nc.scalar if ti == 0 else nc.sync
                        e2.dma_start(out_view[:, t, csl], o[:, osl])
                else:
                    dst = out_view[:, 2 * tp:2 * tp + 2, csl]
                    src_ap = o[:].rearrange("p (two n) -> p two n", two=2)
                    eng.dma_start(dst, src_ap)

            emit_pair(0, 0, split=True)
            emit_pair(0, 1)
            emit_pair(0, 2)
            emit_dbc(1)
            emit_pair(0, 3)
            for tp in range(C // 2):
                emit_pair(1, tp)

    nc.compile()
    return nc


def _get_nc():
    if "nc" not in _CACHE:
        _CACHE["nc"] = _build()
    return _CACHE["nc"]


def build_in_maps(feature_matrix: np.ndarray, attention_vector: np.ndarray):
    feature_matrix = np.ascontiguousarray(feature_matrix, dtype=np.float32)
    attention_vector = np.ascontiguousarray(attention_vector, dtype=np.float32)
    import ml_dtypes
    bf16 = ml_dtypes.bfloat16
    xt = np.ascontiguousarray(feature_matrix.T.astype(bf16))
    av_cols = np.ascontiguousarray(attention_vector.reshape(4, P).T)
    in_maps = []
    for c in range(NCORES):
        shard = feature_matrix[c * ROWS:(c + 1) * ROWS]
        xlocT = np.ascontiguousarray(shard.T.astype(bf16))
        in_maps.append({"xt": xt, "xlocT": xlocT, "av_cols": av_cols})
    return in_maps


def kernel(feature_matrix: np.ndarray, attention_vector: np.ndarray) -> np.ndarray:
    from concourse.bass_utils import run_bass_kernel_spmd

    nc = _get_nc()
    in_maps = build_in_maps(feature_matrix, attention_vector)
    res = run_bass_kernel_spmd(nc, in_maps, core_ids=list(range(NCORES)))
    out = np.concatenate(
        [res.results[c]["out"] for c in range(NCORES)], axis=0
    )
    return out.astype(np.float32)


# revision 51
# speedup vs baseline: 1.0630x; 1.0630x over previous
"""Trainium2 Bass kernel for GAT-style attention score computation.

Math (see reference):
    s_src = X @ a[:F];  s_dst = X @ a[F:]
    e[i, j] = leaky_relu(s_src[i] + s_dst[j], alpha=0.2)

Sharding over 8 NeuronCores: row-shard the N x N output (1024 rows/core).
No collective (a 4 KB AllGather measured ~75 us of latency/skew): each
core receives a replicated bf16 copy of X^T and computes the full s_dst
row vector itself with native-rate bf16 matmuls whose stationary operand
is a_dst replicated along the free dim, so the PSUM result is s_dst
broadcast across all 128 partitions (d_bcast, cast to bf16 in SBUF).

s_src also runs on TensorE: the transposed local shard block
xlocT[:, t*128:(t+1)*128] as stationary against an a_src-replicated
moving tile gives out[p, j] = s_src[t*128+p] for every j -- the
activation bias column for sub-row block t, with no DVE matvec chain.

Main loop (half = 4096 output columns; d_bcast h0 first, h1 built after
the first three h0 pairs): 8 sub-rows t per half, local row r = t*128+p,
paired into 2 MB output DMAs.  Within a pair one row runs on the scalar
engine (Prelu activation with per-partition bias) and one on the vector
engine (u = d+s; v = 0.2*d+0.2*s; max(u,v)), so a pair completes at
max(ACT, DVE).  DVE rows sit only in sync-ring pairs: a scalar-ring
issue would embed a DVE-semaphore wait into the ACT FIFO and stall
activations.  The first pair is split into two 1 MB writes to start the
output stream early.

Output is written in bf16 (harness gate is 2e-2 rel err; bf16 keeps it
~3e-3) and upcast to f32 on the host, halving the dominant HBM write
traffic.  Measured ~70.6 us/core (baseline f32+collective: 238.8 us).
Floor: ~21.3 MB DMA/core at ~400 GB/s + ~7 us boot + ~13 us ramp.
"""

import numpy as np

N = 8192
F = 256
NCORES = 8
ROWS = N // NCORES          # 1024 rows per core
P = 128                     # partitions
C = ROWS // P               # 8 sub-rows per partition
ALPHA = 0.2
HALF = N // 2               # 4096 columns per half
QCH = 512                   # matmul free-dim chunk (one PSUM bank)
NACT = 5                    # sub-rows on the scalar engine (rest on DVE)

_CACHE = {}


def _build():
    import concourse.bacc as bacc
    import concourse.bass as bass
    import concourse.tile as tile
    from concourse import mybir

    fp32 = mybir.dt.float32
    fp16 = mybir.dt.bfloat16  # bf16: TensorE native rate (fp16 matmuls run ~2x slower)

    nc = bacc.Bacc(
        "TRN2",
        target_bir_lowering=False,
        debug=False,
        num_devices=NCORES,
    )

    xt_dram = nc.dram_tensor("xt", [F, N], fp16, kind="ExternalInput")
    # transposed local shard, [F, 1024] fp16 (host-prepared)
    xlt_dram = nc.dram_tensor("xlocT", [F, ROWS], fp16, kind="ExternalInput")
    # av_cols[f, a] = av[a*128 + f] (host-rearranged): cols 0,1 = a_src,
    # cols 2,3 = a_dst
    avc_dram = nc.dram_tensor("av_cols", [P, 4], fp32, kind="ExternalInput")
    out_dram = nc.dram_tensor("out", [ROWS, N], fp16, kind="ExternalOutput")

    with tile.TileContext(nc) as tc:
        with (
            tc.tile_pool(name="const", bufs=1) as const_pool,
            tc.tile_pool(name="xt", bufs=2) as xt_pool,
            tc.tile_pool(name="dbc", bufs=2) as dbc_pool,
            tc.tile_pool(name="uv", bufs=4) as uv_pool,
            tc.tile_pool(name="outp", bufs=5) as out_pool,
            tc.tile_pool(name="psS", bufs=1, space=bass.MemorySpace.PSUM) as psS_pool,
            tc.tile_pool(name="psA", bufs=3, space=bass.MemorySpace.PSUM) as psA_pool,
        ):
            # ---- input loads: one 2 MB DMA per X^T half ([128, 2, 4096]:
            # kb-chunk kb of K in cols [kb*HALF:(kb+1)*HALF]); first half
            # leads on the sync ring, the rest go via the scalar ring ----
            xtAB = [
                xt_pool.tile([P, 2 * HALF], fp16, tag="xtAB", name=f"xtAB{h}")
                for h in range(2)
            ]
            xt_kb = xt_dram.ap().rearrange("(kb p) n -> p kb n", kb=2)
            a_cols = const_pool.tile([P, 4], fp32)
            nc.sync.dma_start(a_cols[:], avc_dram.ap())
            QH = HALF // 2
            xtAB0_v = xtAB[0][:].rearrange("p (kb n) -> p kb n", kb=2)
            nc.sync.dma_start(xtAB0_v[:, :, 0:QH], xt_kb[:, :, 0:QH])
            nc.sync.dma_start(xtAB0_v[:, :, QH:HALF], xt_kb[:, :, QH:HALF])
            xlt = [const_pool.tile([P, ROWS], fp16, name=f"xlt{kb}") for kb in range(2)]
            xlt_ap = xlt_dram.ap()
            for kb in range(2):
                nc.sync.dma_start(xlt[kb][:], xlt_ap[kb * P:(kb + 1) * P, :])
            nc.sync.dma_start(
                xtAB[1][:].rearrange("p (kb n) -> p kb n", kb=2),
                xt_kb[:, :, HALF:N],
            )

            # ---- stationary tiles: a_dst (lhsT) and a_src (rhs) chunks ----
            ones16 = const_pool.tile([P, P], fp16)
            nc.vector.memset(ones16[:], 1.0)
            lhsT = []
            rhs_s = []
            for kb in range(2):
                t_ = const_pool.tile([P, P], fp16, name=f"lhsT{kb}")
                nc.vector.tensor_scalar(
                    t_[:], ones16[:], a_cols[:, 2 + kb:3 + kb], None,
                    op0=mybir.AluOpType.mult,
                )
                lhsT.append(t_)
                r_ = const_pool.tile([P, C], fp16, name=f"rhs_s{kb}")
                nc.vector.tensor_scalar(
                    r_[:], ones16[:, 0:C], a_cols[:, kb:kb + 1], None,
                    op0=mybir.AluOpType.mult,
                )
                rhs_s.append(r_)

            # ---- d_bcast halves (h0 before the s_src matmuls: its input
            # lands first and it gates the main loop), s_dst replicated
            # across partitions ----
            def emit_dbc(h):
                d_ = dbc_pool.tile([P, HALF], fp16, tag="dbc", name=f"dbc{h}")
                dbc.append(d_)
                for q in range(HALF // (2 * QCH)):
                    ps = psA_pool.tile([P, 2 * QCH], fp32, tag="dps")
                    for half_q in range(2):
                        sl_p = slice(half_q * QCH, (half_q + 1) * QCH)
                        sl = slice((2 * q + half_q) * QCH, (2 * q + half_q + 1) * QCH)
                        nc.tensor.matmul(
                            ps[:, sl_p], lhsT[0][:], xtAB[h][:, sl],
                            start=True, stop=False,
                        )
                        nc.tensor.matmul(
                            ps[:, sl_p], lhsT[1][:],
                            xtAB[h][:, HALF + sl.start:HALF + sl.stop],
                            start=False, stop=True,
                        )
                    sl2 = slice(2 * q * QCH, 2 * (q + 1) * QCH)
                    nc.vector.tensor_copy(d_[:, sl2], ps[:])

            dbc = []
            emit_dbc(0)

            # ---- s_src on TensorE: block t of xlocT (stationary) against
            # the a_src-replicated tile -> out[p, j] = s_src[t*128+p] ----
            ps_s = psS_pool.tile([P, C * C], fp32)
            for t in range(C):
                nc.tensor.matmul(
                    ps_s[:, t * C:(t + 1) * C],
                    xlt[0][:, t * P:(t + 1) * P], rhs_s[0][:],
                    start=True, stop=False,
                )
                nc.tensor.matmul(
                    ps_s[:, t * C:(t + 1) * C],
                    xlt[1][:, t * P:(t + 1) * P], rhs_s[1][:],
                    start=False, stop=True,
                )
            s_src = const_pool.tile([P, C * C], fp32)
            nc.vector.tensor_copy(s_src[:], ps_s[:])
            s_srcA = const_pool.tile([P, C * C], fp32)
            nc.gpsimd.tensor_scalar(
                s_srcA[:], s_src[:], ALPHA, None, op0=mybir.AluOpType.mult
            )

            # ---- main loops: sub-rows paired into one 2 MB output DMA;
            # within a pair one row runs on ACT, the other on DVE so the
            # pair completes at max(ACT, DVE), not 2x ACT.  dbc[1] is built
            # after the first two h0 pairs so its PSUM casts don't sit in
            # the DVE FIFO ahead of the first output rows. ----
            # Per-half engine assignment: DVE rows sit only in sync-ring
            # pairs (pairs 0,2 of h0 / 1,3 of h1); scalar-ring pairs are
            # pure-ACT so their DMA issues never stall the ACT FIFO.
            ACT_ROWS_H = [{0, 2, 3, 6, 7}, {0, 2, 3, 6, 7}]
            out_view = out_dram.ap().rearrange("(c p) n -> p c n", c=C)

            def emit_pair(h, tp, split=False):
                ACT_ROWS = ACT_ROWS_H[h]
                csl = slice(h * HALF, (h + 1) * HALF)
                o = out_pool.tile([P, 2 * HALF], fp16)
                for ti in range(2):
                    t = 2 * tp + ti
                    osl = slice(ti * HALF, (ti + 1) * HALF)
                    if t in ACT_ROWS:
                        nc.scalar.activation(
                            o[:, osl],
                            dbc[h][:],
                            mybir.ActivationFunctionType.Prelu,
                            bias=s_src[:, t * C:t * C + 1],
                            scale=1.0,
                            alpha=ALPHA,
                        )
                    else:
                        u = uv_pool.tile([P, HALF], fp16, tag="u")
                        v = uv_pool.tile([P, HALF], fp16, tag="v")
                        nc.vector.tensor_scalar(
                            u[:], dbc[h][:], s_src[:, t * C:t * C + 1], None,
                            op0=mybir.AluOpType.add,
                        )
                        nc.vector.tensor_scalar(
                            v[:], dbc[h][:], ALPHA, s_srcA[:, t * C:t * C + 1],
                            op0=mybir.AluOpType.mult, op1=mybir.AluOpType.add,
                        )
                        nc.vector.tensor_tensor(
                            o[:, osl], u[:], v[:], op=mybir.AluOpType.max
                        )
                eng = nc.sync if (h * 4 + tp) % 2 == 0 else nc.scalar
                if split:
                    for ti in range(2):
                        t = 2 * tp + ti
                        osl = slice(ti * HALF, (ti + 1) * HALF)
                        e2 = nc.scalar if ti == 0 else nc.sync
                        e2.dma_start(out_view[:, t, csl], o[:, osl])
                else:
                    dst = out_view[:, 2 * tp:2 * tp + 2, csl]
                    src_ap = o[:].rearrange("p (two n) -> p two n", two=2)
                    eng.dma_start(dst, src_ap)

            emit_pair(0, 0, split=True)
            emit_pair(0, 1)
            emit_pair(0, 2)
            emit_dbc(1)
            emit_pair(0, 3)
            for tp in range(C // 2):
                emit_pair(1, tp)

    nc.compile()
    return nc


def _get_nc():
    if "nc" not in _CACHE:
        _CACHE["nc"] = _build()
    return _CACHE["nc"]


def build_in_maps(feature_matrix: np.ndarray, attention_vector: np.ndarray):
    feature_matrix = np.ascontiguousarray(feature_matrix, dtype=np.float32)
    attention_vector = np.ascontiguousarray(attention_vector, dtype=np.float32)
    import ml_dtypes
    bf16 = ml_dtypes.bfloat16
    xt = np.ascontiguousarray(feature_matrix.T.astype(bf16))
    av_cols = np.ascontiguousarray(attention_vector.reshape(4, P).T)
    in_maps = []
    for c in range(NCORES):
        shard = feature_matrix[c * ROWS:(c + 1) * ROWS]
        xlocT = np.ascontiguousarray(shard.T.astype(bf16))
        in_maps.append({"xt": xt, "xlocT": xlocT, "av_cols": av_cols})
    return in_maps


def kernel(feature_matrix: np.ndarray, attention_vector: np.ndarray) -> np.ndarray:
    from concourse.bass_utils import run_bass_kernel_spmd

    nc = _get_nc()
    in_maps = build_in_maps(feature_matrix, attention_vector)
    res = run_bass_kernel_spmd(nc, in_maps, core_ids=list(range(NCORES)))
    out = np.concatenate(
        [res.results[c]["out"] for c in range(NCORES)], axis=0
    )
    return out.astype(np.float32)
